# revision 46
# baseline (speedup 1.0000x reference)
"""Trainium2 Bass kernel for nn_CMRLayer (batched 8x1280 rank-1 common-mode removal).

Math (per sample b):
  mu, sigma: row stats of X (8,1280); fn = (X-mu)/sigma
  G = fn fn^T (8x8).  u1 = top eigenvector of G, lam1 = top eigenvalue.
  pc1 = lam1 / (trace(G) + 1e-8); solar_flag from thresholds.
  Y = X - c * sigma*u1 * ((u1/sigma)^T X - alpha), c = sigmoid(gate)*kp_gate

Device algorithm: per-sample Gram of raw X via TensorE (transposed 128-chunks +
accumulating matmuls with an appended ones column producing row sums -> mu),
analytic centering folded into the column-scale matmul group, symmetric scaling
(Ms = S A S / D), 8 matrix squarings (gamma folded into the last copy) + 2
Frobenius-normalized squarings, then a Krylov 2-vector Rayleigh-Ritz (double
Gram-Schmidt, branchless 2x2 eigvec) for u1/lam1. Reconstruction matmuls run in
float32r (fp22 multiply). The per-tile work is software-pipelined in 7 stages
so independent tiles overlap despite in-order engines. Pure data parallel over
8 cores (256 samples each).
"""

import numpy as np

B, S, D = 2048, 8, 1280
NCORES = 8
BPC = B // NCORES              # samples per core = 256
SPT = 16                       # samples per tile (16*8 = 128 partitions)
NT = BPC // SPT                # tiles per core = 16
NSQ = 8                        # plain squarings
NEXTRA = 2                     # normalized squarings
GAMMA = 1e-12
INVD = 1.0 / D
ISQD = float(1.0 / np.sqrt(np.float32(D)))
R1 = np.array([0.75, -0.5, 0.933, 0.25, -0.661, 1.0, 0.831, -0.293], np.float32)

_compiled = {}


def _build_consts():
    ident = np.eye(128, dtype=np.float32)
    maskbd = np.kron(np.eye(16, dtype=np.float32), np.ones((8, 8), np.float32))
    mask16 = np.kron(np.eye(16, dtype=np.float32), np.ones((8, 1), np.float32))
    r1rep = np.tile(R1, 16)[:, None]
    zeros = np.zeros((128, 1), np.float32)
    eps30 = np.full((128, 1), 1e-30, np.float32)
    eps38 = np.full((128, 1), 1e-38, np.float32)
    return np.ascontiguousarray(
        np.concatenate([ident, maskbd, mask16, r1rep, zeros, eps30, eps38],
                       axis=1))  # (128, 276)


def _build_nc(debug=False):
    import concourse.tile as tile
    from concourse import bacc, mybir

    f32 = mybir.dt.float32
    f32r = mybir.dt.float32r
    AF = mybir.ActivationFunctionType
    OP = mybir.AluOpType

    nc = bacc.Bacc("TRN2", target_bir_lowering=False, debug=False)
    x = nc.dram_tensor("x", [NT, 128, D], f32, kind="ExternalInput")
    coef = nc.dram_tensor("coef", [NT, 128, 1], f32, kind="ExternalInput")
    cst = nc.dram_tensor("cst", [128, 276], f32, kind="ExternalInput")
    y = nc.dram_tensor("y", [NT, 128, D], f32, kind="ExternalOutput")
    pc1o = nc.dram_tensor("pc1", [128, NT], f32, kind="ExternalOutput")

    CH = [(0, 512), (512, 512), (1024, 256)]  # free-dim chunks of 1280

    with tile.TileContext(nc) as tc:
        with (
            tc.tile_pool(name="consts", bufs=1) as consts,
            tc.tile_pool(name="xpool", bufs=7) as xpool,
            tc.tile_pool(name="xrp", bufs=7) as xrp,
            tc.tile_pool(name="xta", bufs=3) as xta,
            tc.tile_pool(name="ypool", bufs=3) as ypool,
            tc.tile_pool(name="wvsp", bufs=3) as wvsp,
            tc.tile_pool(name="g128", bufs=3) as g128,
            tc.tile_pool(name="msp", bufs=5) as msp,
            tc.tile_pool(name="pmat", bufs=5) as pmat,
            tc.tile_pool(name="sm", bufs=6) as sm,
            tc.tile_pool(name="pbig", bufs=1, space="PSUM") as pbig,
            tc.tile_pool(name="pg", bufs=1, space="PSUM") as pgp,
            tc.tile_pool(name="psq", bufs=2, space="PSUM") as psq,
            tc.tile_pool(name="psmall", bufs=2, space="PSUM") as psmall,
            tc.tile_pool(name="pw", bufs=2, space="PSUM") as pwp,
        ):
            CST = consts.tile([128, 276], f32)
            nc.sync.dma_start(out=CST, in_=cst[:, :])
            ID = CST[:, 0:128]
            MASKBD = CST[:, 128:256]
            MASK16 = CST[:, 256:272]
            R1REP = CST[:, 272:273]
            nc.const_aps.aps[(f32, 0.0)] = CST[:, 273:274]
            nc.const_aps.aps[(f32, 1e-30)] = CST[:, 274:275]
            nc.const_aps.aps[(f32, 1e-38)] = CST[:, 275:276]
            PC1ALL = consts.tile([128, NT], f32, tag="pc1all")

            st = [dict() for _ in range(NT)]   # per-tile state

            def s0a(t):
                # DMA + transposes + gram
                d = st[t]
                X = xpool.tile([128, D], f32, tag="x")
                nc.sync.dma_start(out=X, in_=x[t, :, :])
                XR = xrp.tile([128, D], f32r, tag="xr")
                nc.gpsimd.dma_start(out=XR, in_=x[t, :, :])
                CN = sm.tile([128, 1], f32, tag="cn")
                nc.gpsimd.dma_start(out=CN, in_=coef[t, :, :])
                d.update(X=X, XR=XR, CN=CN)

                XTA = xta.tile([128, 10, 129], f32, tag="xta")
                nc.gpsimd.memset(XTA[:, :, 128:129], 1.0)
                for gg, (c0, nch) in enumerate([(0, 4), (4, 4), (8, 2)]):
                    PT = pbig.tile([128, nch * 128], f32, tag="pt")
                    for j in range(nch):
                        nc.tensor.transpose(
                            PT[:, j * 128 : (j + 1) * 128],
                            X[:, (c0 + j) * 128 : (c0 + j + 1) * 128],
                            ID,
                        )
                    eng = nc.vector.tensor_copy if gg == 2 else nc.scalar.copy
                    eng(
                        out=XTA[:, c0 : c0 + nch, 0:128],
                        in_=PT[:, : nch * 128].rearrange("p (c k) -> p c k", k=128),
                    )
                PG = pgp.tile([128, 129], f32, tag="pg")
                for c in range(10):
                    nc.tensor.matmul(
                        PG, lhsT=XTA[:, c, 0:128], rhs=XTA[:, c, :],
                        start=(c == 0), stop=(c == 9),
                    )
                PGS = g128.tile([128, 129], f32, tag="pgs")
                nc.vector.tensor_copy(out=PGS, in_=PG)
                d["PGS"] = PGS

            def s0b(t):
                # stats + centering correction + column/row scale -> Ms
                d = st[t]
                PGS = d["PGS"]
                MU = sm.tile([128, 1], f32, tag="mu")
                nc.scalar.mul(out=MU, in_=PGS[:, 128:129], mul=INVD)
                MUNEG = sm.tile([128, 2], f32r, tag="muneg")
                nc.vector.tensor_scalar_mul(
                    out=MUNEG, in0=MU.to_broadcast((128, 2)), scalar1=-1.0)
                A = g128.tile([128, 128], f32, tag="A")
                JUNK = g128.tile([128, 128], f32, tag="junk")
                DIAGD = sm.tile([128, 1], f32, tag="diagd")
                nc.vector.scalar_tensor_tensor(
                    out=JUNK, in0=PGS[:, 0:128], scalar=1.0, in1=ID,
                    op0=OP.mult, op1=OP.mult, accum_out=DIAGD,
                )
                nc.vector.tensor_mul(out=A, in0=PGS[:, 0:128], in1=MASKBD)
                MUD = sm.tile([128, 1], f32, tag="mud")
                nc.vector.tensor_scalar_mul(out=MUD, in0=MU, scalar1=-float(D))
                DIAGC = sm.tile([128, 1], f32, tag="diagc")
                nc.vector.scalar_tensor_tensor(
                    out=DIAGC, in0=MUD, scalar=MU, in1=DIAGD,
                    op0=OP.mult, op1=OP.add,
                )
                SIG = sm.tile([128, 1], f32, tag="sig")
                nc.scalar.activation(out=SIG, in_=DIAGC, func=AF.Sqrt, scale=INVD)
                INV = sm.tile([128, 1], f32, tag="inv")
                nc.vector.reciprocal(out=INV, in_=SIG)
                SC = sm.tile([128, 1], f32, tag="sc")
                nc.vector.tensor_scalar_mul(out=SC, in0=INV, scalar1=ISQD)
                TP = sm.tile([128, 1], f32, tag="tp")
                nc.vector.tensor_scalar(
                    out=TP, in0=DIAGC, scalar1=INV, scalar2=INV,
                    op0=OP.mult, op1=OP.mult,
                )
                MD2A = sm.tile([128, 16], f32, tag="md2a")
                nc.gpsimd.tensor_scalar_mul(out=MD2A, in0=MASK16, scalar1=MU)
                MSCD = sm.tile([128, 1], f32, tag="mscd")
                nc.vector.tensor_scalar(
                    out=MSCD, in0=MU, scalar1=SC, scalar2=-float(D),
                    op0=OP.mult, op1=OP.mult,
                )
                MD2B = sm.tile([128, 16], f32, tag="md2b")
                nc.gpsimd.tensor_scalar_mul(out=MD2B, in0=MASK16, scalar1=MSCD)
                PMA = psmall.tile([16, 128], f32, tag="ps")
                nc.tensor.transpose(PMA, MD2A, ID)
                MDTA = sm.tile([16, 128], f32, tag="mdta")
                nc.vector.tensor_copy(out=MDTA, in_=PMA)
                PMB = psmall.tile([16, 128], f32, tag="ps")
                nc.tensor.transpose(PMB, MD2B, ID)
                MDTB = sm.tile([16, 128], f32, tag="mdtb")
                nc.vector.tensor_copy(out=MDTB, in_=PMB)
                DMAT = g128.tile([128, 128], f32, tag="dmat")
                nc.gpsimd.tensor_scalar_mul(out=DMAT, in0=ID, scalar1=SC)
                PB = psmall.tile([128, 128], f32, tag="ps")
                nc.tensor.matmul(PB, lhsT=A, rhs=DMAT, start=True, stop=False)
                nc.tensor.matmul(PB, lhsT=MDTA, rhs=MDTB, start=False, stop=True)
                MS = msp.tile([128, 128], f32, tag="ms")
                nc.vector.tensor_scalar_mul(out=MS, in0=PB, scalar1=SC)
                d.update(MS=MS, MUNEG=MUNEG, SIG=SIG, INV=INV, TP=TP)

            def _sq_round(cur, idx):
                PS = psq.tile([128, 128], f32, tag="sq")
                nc.tensor.matmul(PS, lhsT=cur, rhs=cur, start=True, stop=True)
                nxt = pmat.tile([128, 128], f32, tag="p")
                if idx == NSQ - 1:
                    nc.scalar.mul(out=nxt, in_=PS, mul=GAMMA)
                elif idx % 3 == 0:
                    nc.vector.tensor_copy(out=nxt, in_=PS)
                else:
                    nc.scalar.copy(out=nxt, in_=PS)
                return nxt

            def s1(t):
                d = st[t]
                cur = d["MS"]
                for i in range(4):
                    cur = _sq_round(cur, i)
                d["cur"] = cur

            def s2a(t):
                d = st[t]
                cur = d["cur"]
                for i in range(4, NSQ):
                    cur = _sq_round(cur, i)
                d["cur"] = cur

            def s2(t):
                d = st[t]
                cur = d["cur"]
                for ex_i in range(NEXTRA):
                    JUNK2 = g128.tile([128, 128], f32, tag="junk2")
                    RS = sm.tile([128, 1], f32, tag="rs")
                    nc.vector.scalar_tensor_tensor(
                        out=JUNK2, in0=cur, scalar=1.0, in1=cur,
                        op0=OP.mult, op1=OP.mult, accum_out=RS,
                    )
                    PF = psmall.tile([128, 8], f32, tag="ps")
                    nc.tensor.matmul(PF[:, 0:1], lhsT=MASKBD, rhs=RS,
                                     start=True, stop=True)
                    FR = sm.tile([128, 1], f32, tag="fr")
                    nc.scalar.activation(out=FR, in_=PF[:, 0:1], func=AF.Sqrt,
                                         bias=1e-30)
                    RSI = sm.tile([128, 1], f32, tag="rsi")
                    nc.vector.reciprocal(out=RSI, in_=FR)
                    PN = pmat.tile([128, 128], f32, tag="p")
                    nc.gpsimd.tensor_scalar_mul(out=PN, in0=cur, scalar1=RSI)
                    PS = psq.tile([128, 128], f32, tag="sq")
                    nc.tensor.matmul(PS, lhsT=PN, rhs=PN, start=True, stop=True)
                    nxt = pmat.tile([128, 128], f32, tag="p")
                    if ex_i % 2 == 0:
                        nc.scalar.copy(out=nxt, in_=PS)
                    else:
                        nc.vector.tensor_copy(out=nxt, in_=PS)
                    cur = nxt
                d["cur"] = cur

            def s3(t):
                d = st[t]
                cur = d["cur"]
                MS = d["MS"]
                V12 = sm.tile([128, 2], f32, tag="v12")
                PV = psmall.tile([128, 8], f32, tag="ps")
                nc.tensor.matmul(PV[:, 0:1], lhsT=cur, rhs=R1REP,
                                 start=True, stop=True)
                nc.vector.tensor_copy(out=V12[:, 0:1], in_=PV[:, 0:1])
                nc.tensor.matmul(PV[:, 1:2], lhsT=MS, rhs=V12[:, 0:1],
                                 start=True, stop=True)
                nc.vector.tensor_copy(out=V12[:, 1:2], in_=PV[:, 1:2])

                PR = sm.tile([128, 3], f32, tag="pr")
                nc.vector.tensor_mul(out=PR[:, 0:1], in0=V12[:, 0:1], in1=V12[:, 0:1])
                nc.vector.tensor_mul(out=PR[:, 1:2], in0=V12[:, 0:1], in1=V12[:, 1:2])
                nc.vector.tensor_mul(out=PR[:, 2:3], in0=V12[:, 1:2], in1=V12[:, 1:2])
                PD = psmall.tile([128, 8], f32, tag="ps")
                nc.tensor.matmul(PD[:, 0:3], lhsT=MASKBD, rhs=PR, start=True, stop=True)
                D22S = sm.tile([128, 1], f32, tag="d22s")
                nc.vector.tensor_copy(out=D22S, in_=PD[:, 2:3])

                SD = sm.tile([128, 1], f32, tag="sd")
                nc.scalar.activation(out=SD, in_=PD[:, 0:1], func=AF.Sqrt, bias=1e-38)
                RQ1 = sm.tile([128, 1], f32, tag="rq1")
                nc.vector.reciprocal(out=RQ1, in_=SD)
                Q1W = sm.tile([128, 2], f32, tag="q1w")
                nc.vector.tensor_scalar_mul(out=Q1W[:, 0:1], in0=V12[:, 0:1],
                                            scalar1=RQ1)
                RQSQ = sm.tile([128, 1], f32, tag="rqsq")
                nc.vector.tensor_mul(out=RQSQ, in0=RQ1, in1=RQ1)
                T2N = sm.tile([128, 1], f32, tag="t2n")
                nc.vector.tensor_scalar(
                    out=T2N, in0=PD[:, 1:2], scalar1=RQSQ, scalar2=-1.0,
                    op0=OP.mult, op1=OP.mult,
                )
                WV = sm.tile([128, 1], f32, tag="wv")
                nc.vector.scalar_tensor_tensor(
                    out=WV, in0=V12[:, 0:1], scalar=T2N, in1=V12[:, 1:2],
                    op0=OP.mult, op1=OP.add,
                )
                PGM = sm.tile([128, 1], f32, tag="pgm")
                nc.vector.tensor_mul(out=PGM, in0=Q1W[:, 0:1], in1=WV)
                PG2 = psmall.tile([128, 8], f32, tag="ps")
                nc.tensor.matmul(PG2[:, 0:1], lhsT=MASKBD, rhs=PGM,
                                 start=True, stop=True)
                GN = sm.tile([128, 1], f32, tag="gn")
                nc.scalar.mul(out=GN, in_=PG2[:, 0:1], mul=-1.0)
                nc.vector.scalar_tensor_tensor(
                    out=Q1W[:, 1:2], in0=Q1W[:, 0:1], scalar=GN, in1=WV,
                    op0=OP.mult, op1=OP.add,
                )

                PZ = psmall.tile([128, 8], f32, tag="ps")
                nc.tensor.matmul(PZ[:, 0:2], lhsT=MS, rhs=Q1W, start=True, stop=True)
                PR3 = sm.tile([128, 5], f32, tag="pr3")
                nc.vector.tensor_mul(out=PR3[:, 0:1], in0=Q1W[:, 1:2], in1=Q1W[:, 1:2])
                nc.vector.tensor_mul(out=PR3[:, 1:2], in0=Q1W[:, 0:1], in1=PZ[:, 0:1])
                nc.vector.tensor_mul(out=PR3[:, 2:3], in0=Q1W[:, 0:1], in1=PZ[:, 1:2])
                nc.vector.tensor_mul(out=PR3[:, 3:4], in0=Q1W[:, 1:2], in1=PZ[:, 1:2])
                nc.vector.tensor_copy(out=PR3[:, 4:5], in_=d["TP"])
                PE5 = psmall.tile([128, 8], f32, tag="ps")
                nc.tensor.matmul(PE5[:, 0:5], lhsT=MASKBD, rhs=PR3,
                                 start=True, stop=True)

                EREG = sm.tile([128, 1], f32, tag="ereg")
                nc.vector.scalar_tensor_tensor(
                    out=EREG, in0=D22S, scalar=1e-16, in1=PE5[:, 0:1],
                    op0=OP.mult, op1=OP.add,
                )
                SQE = sm.tile([128, 1], f32, tag="sqe")
                nc.scalar.activation(out=SQE, in_=EREG, func=AF.Sqrt, bias=1e-38)
                REN = sm.tile([128, 1], f32, tag="ren")
                nc.vector.reciprocal(out=REN, in_=SQE)
                M12 = sm.tile([128, 1], f32, tag="m12")
                nc.vector.tensor_scalar_mul(out=M12, in0=PE5[:, 2:3], scalar1=REN)
                M22 = sm.tile([128, 1], f32, tag="m22")
                nc.vector.tensor_scalar(
                    out=M22, in0=PE5[:, 3:4], scalar1=REN, scalar2=REN,
                    op0=OP.mult, op1=OP.mult,
                )
                M11H = sm.tile([128, 1], f32, tag="m11h")
                nc.vector.tensor_scalar_mul(out=M11H, in0=PE5[:, 1:2], scalar1=0.5)
                H = sm.tile([128, 1], f32, tag="h")
                nc.vector.scalar_tensor_tensor(
                    out=H, in0=M22, scalar=-0.5, in1=M11H, op0=OP.mult, op1=OP.add)
                TM = sm.tile([128, 1], f32, tag="tm")
                nc.vector.tensor_mul(out=TM, in0=M12, in1=M12)
                T2_ = sm.tile([128, 1], f32, tag="t2_")
                nc.vector.scalar_tensor_tensor(
                    out=T2_, in0=H, scalar=H, in1=TM, op0=OP.mult, op1=OP.add)
                SR = sm.tile([128, 1], f32, tag="sr")
                nc.scalar.activation(out=SR, in_=T2_, func=AF.Sqrt)
                LAM = sm.tile([128, 1], f32, tag="lam")
                nc.vector.scalar_tensor_tensor(
                    out=LAM, in0=M22, scalar=0.5, in1=M11H, op0=OP.mult, op1=OP.add)
                nc.vector.tensor_add(out=LAM, in0=LAM, in1=SR)
                D1 = sm.tile([128, 1], f32, tag="d1")
                nc.vector.tensor_add(out=D1, in0=H, in1=SR)
                B1 = sm.tile([128, 1], f32, tag="b1")
                nc.vector.scalar_tensor_tensor(
                    out=B1, in0=D1, scalar=D1, in1=TM, op0=OP.mult, op1=OP.add)
                B2 = sm.tile([128, 1], f32, tag="b2")
                nc.vector.tensor_scalar(
                    out=B2, in0=M12, scalar1=SR, scalar2=2.0, op0=OP.mult, op1=OP.mult)
                QB = sm.tile([128, 1], f32, tag="qb")
                nc.vector.tensor_scalar_mul(out=QB, in0=Q1W[:, 0:1], scalar1=B1)
                T3 = sm.tile([128, 1], f32, tag="t3")
                nc.vector.tensor_scalar(
                    out=T3, in0=Q1W[:, 1:2], scalar1=REN, scalar2=B2,
                    op0=OP.mult, op1=OP.mult,
                )
                UN = sm.tile([128, 1], f32, tag="un")
                nc.vector.tensor_add(out=UN, in0=QB, in1=T3)
                TB2 = sm.tile([128, 1], f32, tag="tb2")
                nc.vector.tensor_mul(out=TB2, in0=B2, in1=B2)
                NN = sm.tile([128, 1], f32, tag="nn")
                nc.vector.scalar_tensor_tensor(
                    out=NN, in0=B1, scalar=B1, in1=TB2, op0=OP.mult, op1=OP.add)
                SN = sm.tile([128, 1], f32, tag="sn")
                nc.scalar.activation(out=SN, in_=NN, func=AF.Sqrt, bias=1e-38)
                RSN = sm.tile([128, 1], f32, tag="rsn")
                nc.vector.reciprocal(out=RSN, in_=SN)
                U = sm.tile([128, 1], f32, tag="u")
                nc.vector.tensor_scalar_mul(out=U, in0=UN, scalar1=RSN)
                TRE = sm.tile([128, 1], f32, tag="tre")
                nc.vector.tensor_scalar_add(out=TRE, in0=PE5[:, 4:5], scalar1=1e-8)
                RT = sm.tile([128, 1], f32, tag="rt")
                nc.vector.reciprocal(out=RT, in_=TRE)
                nc.vector.tensor_scalar(
                    out=PC1ALL[:, t : t + 1], in0=LAM, scalar1=RT, scalar2=float(D),
                    op0=OP.mult, op1=OP.mult,
                )
                d["U"] = U

            def s4(t):
                d = st[t]
                X, XR, U, INV, SIG, CN = (d["X"], d["XR"], d["U"], d["INV"],
                                          d["SIG"], d["CN"])
                UT = sm.tile([128, 1], f32, tag="ut")
                nc.vector.tensor_mul(out=UT, in0=U, in1=INV)
                UD = sm.tile([128, 16], f32r, tag="ud")
                nc.gpsimd.tensor_scalar_mul(out=UD, in0=MASK16, scalar1=UT)
                CU = sm.tile([128, 1], f32, tag="cu")
                nc.vector.tensor_scalar(
                    out=CU, in0=U, scalar1=SIG, scalar2=CN, op0=OP.mult, op1=OP.mult)
                UD2 = sm.tile([128, 16], f32, tag="ud2")
                nc.gpsimd.tensor_scalar_mul(out=UD2, in0=MASK16, scalar1=CU)
                PU2 = psmall.tile([16, 128], f32, tag="ps")
                nc.tensor.transpose(PU2, UD2, ID)
                U2T = sm.tile([16, 128], f32r, tag="u2t")
                nc.vector.tensor_copy(out=U2T, in_=PU2)
                PAN = psmall.tile([16, 8], f32, tag="ps")
                nc.tensor.matmul(PAN[:, 0:2], lhsT=UD, rhs=d["MUNEG"],
                                 start=True, stop=True)
                ANEG = sm.tile([16, 1], f32, tag="aneg")
                nc.vector.tensor_copy(out=ANEG, in_=PAN[:, 0:1])

                WVS = wvsp.tile([16, D], f32r, tag="wvs")
                for ci, (c0, cl) in enumerate(CH):
                    PW = pwp.tile([16, 512], f32, tag="wy")
                    nc.tensor.matmul(
                        PW[:, 0:cl], lhsT=UD, rhs=XR[:, c0 : c0 + cl],
                        start=True, stop=True,
                    )
                    nc.scalar.add(out=WVS[:, c0 : c0 + cl], in_=PW[:, 0:cl],
                                  add=ANEG)

                Y = ypool.tile([128, D], f32, tag="y")
                for ci, (c0, cl) in enumerate(CH):
                    PY = pwp.tile([128, cl], f32, tag="wy")
                    nc.tensor.matmul(PY, lhsT=U2T, rhs=WVS[:, c0 : c0 + cl],
                                     start=True, stop=True)
                    nc.vector.scalar_tensor_tensor(
                        out=Y[:, c0 : c0 + cl], in0=PY, scalar=1.0,
                        in1=X[:, c0 : c0 + cl], op0=OP.mult, op1=OP.add,
                    )
                nc.sync.dma_start(out=y[t, :, :], in_=Y)
                st[t] = {}

            stages = [s0a, s0b, s1, s2a, s2, s3, s4]
            nst = len(stages)
            for step in range(NT + nst - 1):
                for k in range(nst - 1, -1, -1):
                    t = step - k
                    if 0 <= t < NT:
                        stages[k](t)

            nc.sync.dma_start(out=pc1o[:, :], in_=PC1ALL)
    nc.compile()
    return nc


def _get_nc():
    if "nc" not in _compiled:
        _compiled["nc"] = _build_nc()
    return _compiled["nc"]


def kernel(features, kp_index, cmr_gate):
    from concourse.bass_utils import run_bass_kernel_spmd

    features = np.ascontiguousarray(np.asarray(features, np.float32))
    kp_index = np.asarray(kp_index, np.float32)
    cmr_gate = np.asarray(cmr_gate, np.float32)

    gate = 1.0 / (1.0 + np.exp(-float(cmr_gate[0])))
    kpg = 1.0 + 0.5 * (kp_index >= 5.0).astype(np.float32)
    coefneg = (-gate * kpg).astype(np.float32)          # (B,)
    coefrep = np.repeat(coefneg, S)                      # (B*8,)

    cstn = _build_consts()
    nc = _get_nc()

    in_maps = []
    for c in range(NCORES):
        xs = features[c * BPC : (c + 1) * BPC].reshape(NT, 128, D)
        cs = coefrep[c * BPC * S : (c + 1) * BPC * S].reshape(NT, 128, 1)
        in_maps.append({
            "x": np.ascontiguousarray(xs),
            "coef": np.ascontiguousarray(cs),
            "cst": cstn,
        })

    res = run_bass_kernel_spmd(nc, in_maps, core_ids=list(range(NCORES)))
    results = res.results

    yout = np.empty((B, S, D), np.float32)
    pc1 = np.empty((B,), np.float32)
    for c in range(NCORES):
        yout[c * BPC : (c + 1) * BPC] = results[c]["y"].reshape(BPC, S, D)
        pr = results[c]["pc1"].reshape(16, 8, NT)[:, 0, :]   # (16 samples, NT)
        pc1[c * BPC : (c + 1) * BPC] = pr.T.reshape(BPC)

    solar_flag = (pc1 >= 0.6).astype(np.int32) + (pc1 >= 0.8).astype(np.int32)
    return yout, pc1, solar_flag


# revision 49
# speedup vs baseline: 1.0026x; 1.0026x over previous
"""Trainium2 Bass kernel for nn_CMRLayer (batched 8x1280 rank-1 common-mode removal).

Math (per sample b):
  mu, sigma: row stats of X (8,1280); fn = (X-mu)/sigma
  G = fn fn^T (8x8).  u1 = top eigenvector of G, lam1 = top eigenvalue.
  pc1 = lam1 / (trace(G) + 1e-8); solar_flag from thresholds.
  Y = X - c * sigma*u1 * ((u1/sigma)^T X - alpha), c = sigmoid(gate)*kp_gate

Device algorithm: per-sample Gram of raw X via TensorE (transposed 128-chunks +
accumulating matmuls with an appended ones column producing row sums -> mu),
analytic centering folded into the column-scale matmul group, symmetric scaling
(Ms = S A S / D), 8 matrix squarings (gamma folded into the last copy) + 2
Frobenius-normalized squarings, then a Krylov 2-vector Rayleigh-Ritz (double
Gram-Schmidt, branchless 2x2 eigvec) for u1/lam1. Reconstruction matmuls run in
float32r (fp22 multiply). The per-tile work is software-pipelined in 7 stages
so independent tiles overlap despite in-order engines. Pure data parallel over
8 cores (256 samples each).
"""

import numpy as np

B, S, D = 2048, 8, 1280
NCORES = 8
BPC = B // NCORES              # samples per core = 256
SPT = 16                       # samples per tile (16*8 = 128 partitions)
NT = BPC // SPT                # tiles per core = 16
NSQ = 8                        # plain squarings
NEXTRA = 2                     # normalized squarings
GAMMA = 1e-12
INVD = 1.0 / D
ISQD = float(1.0 / np.sqrt(np.float32(D)))
R1 = np.array([0.75, -0.5, 0.933, 0.25, -0.661, 1.0, 0.831, -0.293], np.float32)

_compiled = {}


def _build_consts():
    ident = np.eye(128, dtype=np.float32)
    maskbd = np.kron(np.eye(16, dtype=np.float32), np.ones((8, 8), np.float32))
    mask16 = np.kron(np.eye(16, dtype=np.float32), np.ones((8, 1), np.float32))
    r1rep = np.tile(R1, 16)[:, None]
    zeros = np.zeros((128, 1), np.float32)
    eps30 = np.full((128, 1), 1e-30, np.float32)
    eps38 = np.full((128, 1), 1e-38, np.float32)
    return np.ascontiguousarray(
        np.concatenate([ident, maskbd, mask16, r1rep, zeros, eps30, eps38],
                       axis=1))  # (128, 276)


def _build_nc(debug=False):
    import concourse.tile as tile
    from concourse import bacc, mybir

    f32 = mybir.dt.float32
    f32r = mybir.dt.float32r
    AF = mybir.ActivationFunctionType
    OP = mybir.AluOpType

    nc = bacc.Bacc("TRN2", target_bir_lowering=False, debug=False)
    x = nc.dram_tensor("x", [NT, 128, D], f32, kind="ExternalInput")
    coef = nc.dram_tensor("coef", [NT, 128, 1], f32, kind="ExternalInput")
    cst = nc.dram_tensor("cst", [128, 276], f32, kind="ExternalInput")
    y = nc.dram_tensor("y", [NT, 128, D], f32, kind="ExternalOutput")
    pc1o = nc.dram_tensor("pc1", [128, NT], f32, kind="ExternalOutput")

    CH = [(0, 512), (512, 512), (1024, 256)]  # free-dim chunks of 1280

    with tile.TileContext(nc) as tc:
        with (
            tc.tile_pool(name="consts", bufs=1) as consts,
            tc.tile_pool(name="xpool", bufs=7) as xpool,
            tc.tile_pool(name="xrp", bufs=7) as xrp,
            tc.tile_pool(name="xta", bufs=3) as xta,
            tc.tile_pool(name="ypool", bufs=3) as ypool,
            tc.tile_pool(name="wvsp", bufs=3) as wvsp,
            tc.tile_pool(name="g128", bufs=3) as g128,
            tc.tile_pool(name="msp", bufs=5) as msp,
            tc.tile_pool(name="pmat", bufs=5) as pmat,
            tc.tile_pool(name="sm", bufs=6) as sm,
            tc.tile_pool(name="pbig", bufs=1, space="PSUM") as pbig,
            tc.tile_pool(name="pg", bufs=1, space="PSUM") as pgp,
            tc.tile_pool(name="psq", bufs=2, space="PSUM") as psq,
            tc.tile_pool(name="psmall", bufs=2, space="PSUM") as psmall,
            tc.tile_pool(name="pw", bufs=2, space="PSUM") as pwp,
        ):
            CST = consts.tile([128, 276], f32)
            nc.sync.dma_start(out=CST, in_=cst[:, :])
            ID = CST[:, 0:128]
            MASKBD = CST[:, 128:256]
            MASK16 = CST[:, 256:272]
            R1REP = CST[:, 272:273]
            nc.const_aps.aps[(f32, 0.0)] = CST[:, 273:274]
            nc.const_aps.aps[(f32, 1e-30)] = CST[:, 274:275]
            nc.const_aps.aps[(f32, 1e-38)] = CST[:, 275:276]
            PC1ALL = consts.tile([128, NT], f32, tag="pc1all")

            st = [dict() for _ in range(NT)]   # per-tile state

            def s0a(t):
                # DMA + transposes + gram
                d = st[t]
                X = xpool.tile([128, D], f32, tag="x")
                nc.sync.dma_start(out=X, in_=x[t, :, :])
                XR = xrp.tile([128, D], f32r, tag="xr")
                nc.gpsimd.dma_start(out=XR, in_=x[t, :, :])
                CN = sm.tile([128, 1], f32, tag="cn")
                nc.gpsimd.dma_start(out=CN, in_=coef[t, :, :])
                d.update(X=X, XR=XR, CN=CN)

                XTA = xta.tile([128, 10, 129], f32, tag="xta")
                nc.gpsimd.memset(XTA[:, :, 128:129], 1.0)
                for gg, (c0, nch) in enumerate([(0, 4), (4, 4), (8, 2)]):
                    PT = pbig.tile([128, nch * 128], f32, tag="pt")
                    for j in range(nch):
                        nc.tensor.transpose(
                            PT[:, j * 128 : (j + 1) * 128],
                            X[:, (c0 + j) * 128 : (c0 + j + 1) * 128],
                            ID,
                        )
                    eng = nc.vector.tensor_copy if gg == 2 else nc.scalar.copy
                    eng(
                        out=XTA[:, c0 : c0 + nch, 0:128],
                        in_=PT[:, : nch * 128].rearrange("p (c k) -> p c k", k=128),
                    )
                PG = pgp.tile([128, 129], f32, tag="pg")
                for c in range(10):
                    nc.tensor.matmul(
                        PG, lhsT=XTA[:, c, 0:128], rhs=XTA[:, c, :],
                        start=(c == 0), stop=(c == 9),
                    )
                PGS = g128.tile([128, 129], f32, tag="pgs")
                nc.vector.tensor_copy(out=PGS, in_=PG)
                d["PGS"] = PGS

            def s0b(t):
                # stats + centering correction + column/row scale -> Ms
                d = st[t]
                PGS = d["PGS"]
                MU = sm.tile([128, 1], f32, tag="mu")
                nc.scalar.mul(out=MU, in_=PGS[:, 128:129], mul=INVD)
                MUNEG = sm.tile([128, 2], f32r, tag="muneg")
                nc.vector.tensor_scalar_mul(
                    out=MUNEG, in0=MU.to_broadcast((128, 2)), scalar1=-1.0)
                A = g128.tile([128, 128], f32, tag="A")
                JUNK = g128.tile([128, 128], f32, tag="junk")
                DIAGD = sm.tile([128, 1], f32, tag="diagd")
                nc.vector.scalar_tensor_tensor(
                    out=JUNK, in0=PGS[:, 0:128], scalar=1.0, in1=ID,
                    op0=OP.mult, op1=OP.mult, accum_out=DIAGD,
                )
                nc.vector.tensor_mul(out=A, in0=PGS[:, 0:128], in1=MASKBD)
                MUD = sm.tile([128, 1], f32, tag="mud")
                nc.vector.tensor_scalar_mul(out=MUD, in0=MU, scalar1=-float(D))
                DIAGC = sm.tile([128, 1], f32, tag="diagc")
                nc.vector.scalar_tensor_tensor(
                    out=DIAGC, in0=MUD, scalar=MU, in1=DIAGD,
                    op0=OP.mult, op1=OP.add,
                )
                SIG0 = sm.tile([128, 1], f32, tag="sig0")
                nc.scalar.activation(out=SIG0, in_=DIAGC, func=AF.Sqrt, scale=INVD)
                # one Newton step: sig = 0.5*(sig0 + (diagc/D)/sig0)  (ACT sqrt
                # has a loose ULP budget; sigma feeds Y directly)
                R0 = sm.tile([128, 1], f32, tag="r0")
                nc.vector.reciprocal(out=R0, in_=SIG0)
                VT = sm.tile([128, 1], f32, tag="vt")
                nc.vector.tensor_scalar(
                    out=VT, in0=DIAGC, scalar1=R0, scalar2=0.5 * INVD,
                    op0=OP.mult, op1=OP.mult,
                )
                SIG = sm.tile([128, 1], f32, tag="sig")
                nc.vector.scalar_tensor_tensor(
                    out=SIG, in0=SIG0, scalar=0.5, in1=VT, op0=OP.mult, op1=OP.add)
                INV = sm.tile([128, 1], f32, tag="inv")
                nc.vector.reciprocal(out=INV, in_=SIG)
                SC = sm.tile([128, 1], f32, tag="sc")
                nc.vector.tensor_scalar_mul(out=SC, in0=INV, scalar1=ISQD)
                TP = sm.tile([128, 1], f32, tag="tp")
                nc.vector.tensor_scalar(
                    out=TP, in0=DIAGC, scalar1=INV, scalar2=INV,
                    op0=OP.mult, op1=OP.mult,
                )
                MD2A = sm.tile([128, 16], f32, tag="md2a")
                nc.gpsimd.tensor_scalar_mul(out=MD2A, in0=MASK16, scalar1=MU)
                MSCD = sm.tile([128, 1], f32, tag="mscd")
                nc.vector.tensor_scalar(
                    out=MSCD, in0=MU, scalar1=SC, scalar2=-float(D),
                    op0=OP.mult, op1=OP.mult,
                )
                MD2B = sm.tile([128, 16], f32, tag="md2b")
                nc.gpsimd.tensor_scalar_mul(out=MD2B, in0=MASK16, scalar1=MSCD)
                PMA = psmall.tile([16, 128], f32, tag="ps")
                nc.tensor.transpose(PMA, MD2A, ID)
                MDTA = sm.tile([16, 128], f32, tag="mdta")
                nc.vector.tensor_copy(out=MDTA, in_=PMA)
                PMB = psmall.tile([16, 128], f32, tag="ps")
                nc.tensor.transpose(PMB, MD2B, ID)
                MDTB = sm.tile([16, 128], f32, tag="mdtb")
                nc.vector.tensor_copy(out=MDTB, in_=PMB)
                DMAT = g128.tile([128, 128], f32, tag="dmat")
                nc.gpsimd.tensor_scalar_mul(out=DMAT, in0=ID, scalar1=SC)
                PB = psmall.tile([128, 128], f32, tag="ps")
                nc.tensor.matmul(PB, lhsT=A, rhs=DMAT, start=True, stop=False)
                nc.tensor.matmul(PB, lhsT=MDTA, rhs=MDTB, start=False, stop=True)
                MS = msp.tile([128, 128], f32, tag="ms")
                nc.vector.tensor_scalar_mul(out=MS, in0=PB, scalar1=SC)
                d.update(MS=MS, MUNEG=MUNEG, SIG=SIG, INV=INV, TP=TP)

            def _sq_round(cur, idx):
                PS = psq.tile([128, 128], f32, tag="sq")
                nc.tensor.matmul(PS, lhsT=cur, rhs=cur, start=True, stop=True)
                nxt = pmat.tile([128, 128], f32, tag="p")
                if idx == NSQ - 1:
                    nc.scalar.mul(out=nxt, in_=PS, mul=GAMMA)
                elif idx % 3 == 0:
                    nc.vector.tensor_copy(out=nxt, in_=PS)
                else:
                    nc.scalar.copy(out=nxt, in_=PS)
                return nxt

            def s1(t):
                d = st[t]
                cur = d["MS"]
                for i in range(4):
                    cur = _sq_round(cur, i)
                d["cur"] = cur

            def s2a(t):
                d = st[t]
                cur = d["cur"]
                for i in range(4, NSQ):
                    cur = _sq_round(cur, i)
                d["cur"] = cur

            def s2(t):
                d = st[t]
                cur = d["cur"]
                for ex_i in range(NEXTRA):
                    JUNK2 = g128.tile([128, 128], f32, tag="junk2")
                    RS = sm.tile([128, 1], f32, tag="rs")
                    nc.vector.scalar_tensor_tensor(
                        out=JUNK2, in0=cur, scalar=1.0, in1=cur,
                        op0=OP.mult, op1=OP.mult, accum_out=RS,
                    )
                    PF = psmall.tile([128, 8], f32, tag="ps")
                    nc.tensor.matmul(PF[:, 0:1], lhsT=MASKBD, rhs=RS,
                                     start=True, stop=True)
                    FR = sm.tile([128, 1], f32, tag="fr")
                    nc.scalar.activation(out=FR, in_=PF[:, 0:1], func=AF.Sqrt,
                                         bias=1e-30)
                    RSI = sm.tile([128, 1], f32, tag="rsi")
                    nc.vector.reciprocal(out=RSI, in_=FR)
                    PN = pmat.tile([128, 128], f32, tag="p")
                    nc.gpsimd.tensor_scalar_mul(out=PN, in0=cur, scalar1=RSI)
                    PS = psq.tile([128, 128], f32, tag="sq")
                    nc.tensor.matmul(PS, lhsT=PN, rhs=PN, start=True, stop=True)
                    nxt = pmat.tile([128, 128], f32, tag="p")
                    if ex_i % 2 == 0:
                        nc.scalar.copy(out=nxt, in_=PS)
                    else:
                        nc.vector.tensor_copy(out=nxt, in_=PS)
                    cur = nxt
                d["cur"] = cur

            def s3(t):
                d = st[t]
                cur = d["cur"]
                MS = d["MS"]
                V12 = sm.tile([128, 2], f32, tag="v12")
                PV = psmall.tile([128, 8], f32, tag="ps")
                nc.tensor.matmul(PV[:, 0:1], lhsT=cur, rhs=R1REP,
                                 start=True, stop=True)
                nc.vector.tensor_copy(out=V12[:, 0:1], in_=PV[:, 0:1])
                nc.tensor.matmul(PV[:, 1:2], lhsT=MS, rhs=V12[:, 0:1],
                                 start=True, stop=True)
                nc.vector.tensor_copy(out=V12[:, 1:2], in_=PV[:, 1:2])

                PR = sm.tile([128, 3], f32, tag="pr")
                nc.vector.tensor_mul(out=PR[:, 0:1], in0=V12[:, 0:1], in1=V12[:, 0:1])
                nc.vector.tensor_mul(out=PR[:, 1:2], in0=V12[:, 0:1], in1=V12[:, 1:2])
                nc.vector.tensor_mul(out=PR[:, 2:3], in0=V12[:, 1:2], in1=V12[:, 1:2])
                PD = psmall.tile([128, 8], f32, tag="ps")
                nc.tensor.matmul(PD[:, 0:3], lhsT=MASKBD, rhs=PR, start=True, stop=True)
                D22S = sm.tile([128, 1], f32, tag="d22s")
                nc.vector.tensor_copy(out=D22S, in_=PD[:, 2:3])

                SD = sm.tile([128, 1], f32, tag="sd")
                nc.scalar.activation(out=SD, in_=PD[:, 0:1], func=AF.Sqrt, bias=1e-38)
                RQ1 = sm.tile([128, 1], f32, tag="rq1")
                nc.vector.reciprocal(out=RQ1, in_=SD)
                Q1W = sm.tile([128, 2], f32, tag="q1w")
                nc.vector.tensor_scalar_mul(out=Q1W[:, 0:1], in0=V12[:, 0:1],
                                            scalar1=RQ1)
                RQSQ = sm.tile([128, 1], f32, tag="rqsq")
                nc.vector.tensor_mul(out=RQSQ, in0=RQ1, in1=RQ1)
                T2N = sm.tile([128, 1], f32, tag="t2n")
                nc.vector.tensor_scalar(
                    out=T2N, in0=PD[:, 1:2], scalar1=RQSQ, scalar2=-1.0,
                    op0=OP.mult, op1=OP.mult,
                )
                WV = sm.tile([128, 1], f32, tag="wv")
                nc.vector.scalar_tensor_tensor(
                    out=WV, in0=V12[:, 0:1], scalar=T2N, in1=V12[:, 1:2],
                    op0=OP.mult, op1=OP.add,
                )
                PGM = sm.tile([128, 1], f32, tag="pgm")
                nc.vector.tensor_mul(out=PGM, in0=Q1W[:, 0:1], in1=WV)
                PG2 = psmall.tile([128, 8], f32, tag="ps")
                nc.tensor.matmul(PG2[:, 0:1], lhsT=MASKBD, rhs=PGM,
                                 start=True, stop=True)
                GN = sm.tile([128, 1], f32, tag="gn")
                nc.scalar.mul(out=GN, in_=PG2[:, 0:1], mul=-1.0)
                nc.vector.scalar_tensor_tensor(
                    out=Q1W[:, 1:2], in0=Q1W[:, 0:1], scalar=GN, in1=WV,
                    op0=OP.mult, op1=OP.add,
                )

                PZ = psmall.tile([128, 8], f32, tag="ps")
                nc.tensor.matmul(PZ[:, 0:2], lhsT=MS, rhs=Q1W, start=True, stop=True)
                PR3 = sm.tile([128, 5], f32, tag="pr3")
                nc.vector.tensor_mul(out=PR3[:, 0:1], in0=Q1W[:, 1:2], in1=Q1W[:, 1:2])
                nc.vector.tensor_mul(out=PR3[:, 1:2], in0=Q1W[:, 0:1], in1=PZ[:, 0:1])
                nc.vector.tensor_mul(out=PR3[:, 2:3], in0=Q1W[:, 0:1], in1=PZ[:, 1:2])
                nc.vector.tensor_mul(out=PR3[:, 3:4], in0=Q1W[:, 1:2], in1=PZ[:, 1:2])
                nc.vector.tensor_copy(out=PR3[:, 4:5], in_=d["TP"])
                PE5 = psmall.tile([128, 8], f32, tag="ps")
                nc.tensor.matmul(PE5[:, 0:5], lhsT=MASKBD, rhs=PR3,
                                 start=True, stop=True)

                EREG = sm.tile([128, 1], f32, tag="ereg")
                nc.vector.scalar_tensor_tensor(
                    out=EREG, in0=D22S, scalar=1e-16, in1=PE5[:, 0:1],
                    op0=OP.mult, op1=OP.add,
                )
                SQE = sm.tile([128, 1], f32, tag="sqe")
                nc.scalar.activation(out=SQE, in_=EREG, func=AF.Sqrt, bias=1e-38)
                REN = sm.tile([128, 1], f32, tag="ren")
                nc.vector.reciprocal(out=REN, in_=SQE)
                M12 = sm.tile([128, 1], f32, tag="m12")
                nc.vector.tensor_scalar_mul(out=M12, in0=PE5[:, 2:3], scalar1=REN)
                M22 = sm.tile([128, 1], f32, tag="m22")
                nc.vector.tensor_scalar(
                    out=M22, in0=PE5[:, 3:4], scalar1=REN, scalar2=REN,
                    op0=OP.mult, op1=OP.mult,
                )
                M11H = sm.tile([128, 1], f32, tag="m11h")
                nc.vector.tensor_scalar_mul(out=M11H, in0=PE5[:, 1:2], scalar1=0.5)
                H = sm.tile([128, 1], f32, tag="h")
                nc.vector.scalar_tensor_tensor(
                    out=H, in0=M22, scalar=-0.5, in1=M11H, op0=OP.mult, op1=OP.add)
                TM = sm.tile([128, 1], f32, tag="tm")
                nc.vector.tensor_mul(out=TM, in0=M12, in1=M12)
                T2_ = sm.tile([128, 1], f32, tag="t2_")
                nc.vector.scalar_tensor_tensor(
                    out=T2_, in0=H, scalar=H, in1=TM, op0=OP.mult, op1=OP.add)
                SR = sm.tile([128, 1], f32, tag="sr")
                nc.scalar.activation(out=SR, in_=T2_, func=AF.Sqrt)
                LAM = sm.tile([128, 1], f32, tag="lam")
                nc.vector.scalar_tensor_tensor(
                    out=LAM, in0=M22, scalar=0.5, in1=M11H, op0=OP.mult, op1=OP.add)
                nc.vector.tensor_add(out=LAM, in0=LAM, in1=SR)
                D1 = sm.tile([128, 1], f32, tag="d1")
                nc.vector.tensor_add(out=D1, in0=H, in1=SR)
                B1 = sm.tile([128, 1], f32, tag="b1")
                nc.vector.scalar_tensor_tensor(
                    out=B1, in0=D1, scalar=D1, in1=TM, op0=OP.mult, op1=OP.add)
                B2 = sm.tile([128, 1], f32, tag="b2")
                nc.vector.tensor_scalar(
                    out=B2, in0=M12, scalar1=SR, scalar2=2.0, op0=OP.mult, op1=OP.mult)
                QB = sm.tile([128, 1], f32, tag="qb")
                nc.vector.tensor_scalar_mul(out=QB, in0=Q1W[:, 0:1], scalar1=B1)
                T3 = sm.tile([128, 1], f32, tag="t3")
                nc.vector.tensor_scalar(
                    out=T3, in0=Q1W[:, 1:2], scalar1=REN, scalar2=B2,
                    op0=OP.mult, op1=OP.mult,
                )
                UN = sm.tile([128, 1], f32, tag="un")
                nc.vector.tensor_add(out=UN, in0=QB, in1=T3)
                TB2 = sm.tile([128, 1], f32, tag="tb2")
                nc.vector.tensor_mul(out=TB2, in0=B2, in1=B2)
                NN = sm.tile([128, 1], f32, tag="nn")
                nc.vector.scalar_tensor_tensor(
                    out=NN, in0=B1, scalar=B1, in1=TB2, op0=OP.mult, op1=OP.add)
                SN = sm.tile([128, 1], f32, tag="sn")
                nc.scalar.activation(out=SN, in_=NN, func=AF.Sqrt, bias=1e-38)
                RSN = sm.tile([128, 1], f32, tag="rsn")
                nc.vector.reciprocal(out=RSN, in_=SN)
                U = sm.tile([128, 1], f32, tag="u")
                nc.vector.tensor_scalar_mul(out=U, in0=UN, scalar1=RSN)
                TRE = sm.tile([128, 1], f32, tag="tre")
                nc.vector.tensor_scalar_add(out=TRE, in0=PE5[:, 4:5], scalar1=1e-8)
                RT = sm.tile([128, 1], f32, tag="rt")
                nc.vector.reciprocal(out=RT, in_=TRE)
                nc.vector.tensor_scalar(
                    out=PC1ALL[:, t : t + 1], in0=LAM, scalar1=RT, scalar2=float(D),
                    op0=OP.mult, op1=OP.mult,
                )
                d["U"] = U

            def s4(t):
                d = st[t]
                X, XR, U, INV, SIG, CN = (d["X"], d["XR"], d["U"], d["INV"],
                                          d["SIG"], d["CN"])
                UT = sm.tile([128, 1], f32, tag="ut")
                nc.vector.tensor_mul(out=UT, in0=U, in1=INV)
                UD = sm.tile([128, 16], f32r, tag="ud")
                nc.gpsimd.tensor_scalar_mul(out=UD, in0=MASK16, scalar1=UT)
                CU = sm.tile([128, 1], f32, tag="cu")
                nc.vector.tensor_scalar(
                    out=CU, in0=U, scalar1=SIG, scalar2=CN, op0=OP.mult, op1=OP.mult)
                UD2 = sm.tile([128, 16], f32, tag="ud2")
                nc.gpsimd.tensor_scalar_mul(out=UD2, in0=MASK16, scalar1=CU)
                PU2 = psmall.tile([16, 128], f32, tag="ps")
                nc.tensor.transpose(PU2, UD2, ID)
                U2T = sm.tile([16, 128], f32r, tag="u2t")
                nc.vector.tensor_copy(out=U2T, in_=PU2)
                PAN = psmall.tile([16, 8], f32, tag="ps")
                nc.tensor.matmul(PAN[:, 0:2], lhsT=UD, rhs=d["MUNEG"],
                                 start=True, stop=True)
                ANEG = sm.tile([16, 1], f32, tag="aneg")
                nc.vector.tensor_copy(out=ANEG, in_=PAN[:, 0:1])

                WVS = wvsp.tile([16, D], f32r, tag="wvs")
                for ci, (c0, cl) in enumerate(CH):
                    PW = pwp.tile([16, 512], f32, tag="wy")
                    nc.tensor.matmul(
                        PW[:, 0:cl], lhsT=UD, rhs=XR[:, c0 : c0 + cl],
                        start=True, stop=True,
                    )
                    nc.scalar.add(out=WVS[:, c0 : c0 + cl], in_=PW[:, 0:cl],
                                  add=ANEG)

                Y = ypool.tile([128, D], f32, tag="y")
                for ci, (c0, cl) in enumerate(CH):
                    PY = pwp.tile([128, cl], f32, tag="wy")
                    nc.tensor.matmul(PY, lhsT=U2T, rhs=WVS[:, c0 : c0 + cl],
                                     start=True, stop=True)
                    nc.vector.scalar_tensor_tensor(
                        out=Y[:, c0 : c0 + cl], in0=PY, scalar=1.0,
                        in1=X[:, c0 : c0 + cl], op0=OP.mult, op1=OP.add,
                    )
                nc.sync.dma_start(out=y[t, :, :], in_=Y)
                st[t] = {}

            stages = [s0a, s0b, s1, s2a, s2, s3, s4]
            nst = len(stages)
            for step in range(NT + nst - 1):
                for k in range(nst - 1, -1, -1):
                    t = step - k
                    if 0 <= t < NT:
                        stages[k](t)

            nc.sync.dma_start(out=pc1o[:, :], in_=PC1ALL)
    nc.compile()
    return nc


def _get_nc():
    if "nc" not in _compiled:
        _compiled["nc"] = _build_nc()
    return _compiled["nc"]


def kernel(features, kp_index, cmr_gate):
    from concourse.bass_utils import run_bass_kernel_spmd

    features = np.ascontiguousarray(np.asarray(features, np.float32))
    kp_index = np.asarray(kp_index, np.float32)
    cmr_gate = np.asarray(cmr_gate, np.float32)

    gate = 1.0 / (1.0 + np.exp(-float(cmr_gate[0])))
    kpg = 1.0 + 0.5 * (kp_index >= 5.0).astype(np.float32)
    coefneg = (-gate * kpg).astype(np.float32)          # (B,)
    coefrep = np.repeat(coefneg, S)                      # (B*8,)

    cstn = _build_consts()
    nc = _get_nc()

    in_maps = []
    for c in range(NCORES):
        xs = features[c * BPC : (c + 1) * BPC].reshape(NT, 128, D)
        cs = coefrep[c * BPC * S : (c + 1) * BPC * S].reshape(NT, 128, 1)
        in_maps.append({
            "x": np.ascontiguousarray(xs),
            "coef": np.ascontiguousarray(cs),
            "cst": cstn,
        })

    res = run_bass_kernel_spmd(nc, in_maps, core_ids=list(range(NCORES)))
    results = res.results

    yout = np.empty((B, S, D), np.float32)
    pc1 = np.empty((B,), np.float32)
    for c in range(NCORES):
        yout[c * BPC : (c + 1) * BPC] = results[c]["y"].reshape(BPC, S, D)
        pr = results[c]["pc1"].reshape(16, 8, NT)[:, 0, :]   # (16 samples, NT)
        pc1[c * BPC : (c + 1) * BPC] = pr.T.reshape(BPC)

    solar_flag = (pc1 >= 0.6).astype(np.int32) + (pc1 >= 0.8).astype(np.int32)
    return yout, pc1, solar_flag
